# revision 14
# baseline (speedup 1.0000x reference)
"""Trainium2 Bass kernel for dynamic-scale FP8 GEMM (MixLinear):

    out = (scale_in * scale_w) * (q8(x / scale_in) @ q8(w).T) + bias
    scale_in = max|x| / 448  (global over the whole activation tensor)

Strategy (8 NeuronCores, SPMD, data-parallel over M = B*S = 16384):

  - NeuronCores start a NEFF up to ~30us apart (runtime dispatch skew); the
    amax AllGather is the one rendezvous, so total = skew + pre-collective
    phase + collective + post phase.  The PRE phase is minimized: x is
    loaded in its natural [m-partition, k] layout as 8 x 1MB contiguous
    DMAs split across the two HWDGE queues (Sync+Scalar), with abs-max
    reduces (DVE XY + GpSimd XYZWC) trailing each piece.  Weight and bias
    loads wait until after the collective doorbell so they don't steal HBM
    bandwidth from the x read.
  - Weight is quantized to fp8 e4m3 ON THE HOST (static scale 1.0 -> plain
    RNE cast; values << 240 so OCP e4m3fn bits == TRN fp8e4 bits), packed
    in k-PAIR order, and loaded with a straight HWDGE DMA.
  - TRN fp8_e4m3 saturates at +-240 (vs OCP e4m3fn's +-448), so x is
    quantized with a 2x scale (values land in +-224) and the 2x folds back
    into the dequant scale.
  - After the collective, x is quantized in NATURAL layout (fp8 [m-part,k])
    and transposed on-chip by viewing adjacent fp8 k-PAIRS as one fp16
    element: a [128m, 1024]-fp16 xbar transpose moves HALF the bytes of an
    fp16 transpose and lands fp8 pairs contiguously.  The DoubleRow GEMM
    reads the pair with a [128, 2(stride 1), 512(stride 2)] moving AP, and
    the host packs the weight rows in matching k-pair order.
  - The per-m-block transposes run DURING the GEMM (they feed it block by
    block); the first two GEMM chunks are 256-m so compute starts after
    only 2 blocks.  Xbar transposes stay on ONE queue and output-eviction
    DMAs are queue-ordered after the last transpose (transpose||copy
    hazard); a deep ob pool absorbs the backlog.
  - PSUM is evicted with a single ScalarE activation: out = psum*2s + bias
    (output N-major: psum partitions = n-tile, bias is a per-partition
    scalar).  Per-core output is [N, M_shard]; the host transposes on
    gather.
"""

import os
import sys

try:
    import concourse  # noqa: F401
except ImportError:  # pragma: no cover
    for _p in ("/opt/trn_rl_repo", "/root/.axon_site/_ro/trn_rl_repo"):
        if os.path.isdir(_p) and _p not in sys.path:
            sys.path.insert(0, _p)

import ml_dtypes
import numpy as np

import concourse.bacc as bacc
import concourse.bass as bass  # noqa: F401
import concourse.mybir as mybir
import concourse.tile as tile
from concourse.bass_utils import run_bass_kernel_spmd

# Problem shapes (hardcoded per contract).
B, S, K, N = 4, 4096, 2048, 2048
M = B * S
N_CORES = 8
MS = M // N_CORES  # 2048 rows of x per core

P = 128
F16 = mybir.dt.float16
F32 = mybir.dt.float32
FP8 = mybir.dt.float8e4

# An early dummy AllGather (CC pre-warm) wedges the device
# (NRT_EXEC_UNIT_UNRECOVERABLE) -- keep off unless experimenting.
WARM_CC = bool(int(os.environ.get("KERNEL_WARMCC", "0")))
# m-block spans (in 128-row blocks) of the GEMM chunks: small lead-in
# chunks so the first matmuls only wait for 2 quant+transpose blocks.
CHUNK_PLAN = [(0, 2), (2, 4), (4, 8), (8, 12), (12, 16)]


def build_nc(ms=MS, k=K, n=N, n_cores=N_CORES):
    """Build + compile the per-core Bass program (SPMD: same NEFF on all cores)."""
    ko = k // P          # k planes (128 each)
    kj = ko // 2         # DoubleRow k steps (256 each)
    mg_n = ms // P       # m blocks (128 rows each)
    nt_tiles = n // P    # GEMM stationary n-tiles
    assert k % 256 == 0 and ms % 512 == 0 and n % 256 == 0
    assert CHUNK_PLAN[-1][1] == mg_n

    nc = bacc.Bacc("TRN2", target_bir_lowering=False, debug=False, num_devices=n_cores)
    x = nc.dram_tensor("x", [ms, k], F16, kind="ExternalInput")
    wq8 = nc.dram_tensor("wq8", [k, n], FP8, kind="ExternalInput")
    b = nc.dram_tensor("b", [P, n // P], F16, kind="ExternalInput")
    out_t = nc.dram_tensor("out_t", [n, ms], F16, kind="ExternalOutput")

    with tile.TileContext(nc) as tc:
        with (
            tc.tile_pool(name="big", bufs=1) as big,
            tc.tile_pool(name="small", bufs=1) as small,
            tc.tile_pool(name="ev", bufs=12) as ev,
            tc.tile_pool(name="psum", bufs=2, space="PSUM") as psum,
            tc.tile_pool(name="dram", bufs=1, space="DRAM") as dram,
        ):
            # Persistent SBUF tensors.
            xnat = big.tile([P, mg_n, k], F16)   # x natural: [p, mg, k] = x[mg*128+p, k]
            xqn = big.tile([P, mg_n, k], FP8)    # quantized x, natural layout
            # packed transpose target: fp16 element [q, jj, m] = fp8 pair
            # (k = 2*(jj*128+q) + {0,1}) of column m
            xqT = big.tile([P, kj, ms], F16)
            wq = big.tile([P, ko, n], FP8)       # w fp8, host k-pair packing

            cc_addr = "Shared" if n_cores > 4 else "Local"
            if WARM_CC:
                # Pre-warm the CC stream: a dummy 4-byte AllGather issued at
                # t~0 pays the collective wakeup cost while the x load runs.
                warm_src = small.tile([P, 1], F32)
                nc.gpsimd.memset(warm_src[0:1, :], 0.0)
                warm_in = dram.tile([1], F32)
                warm_out = dram.tile([n_cores], F32, addr_space=cc_addr)
                nc.scalar.dma_start(warm_in[:], warm_src[0:1, 0])
                nc.gpsimd.collective_compute(
                    "AllGather",
                    mybir.AluOpType.bypass,
                    replica_groups=[list(range(n_cores))],
                    ins=[warm_in.opt()],
                    outs=[warm_out.opt()],
                )

            # ---- Phase A: natural x load + amax -------------------------
            # 8 x 1MB contiguous loads (2 m-blocks each), alternating
            # Sync/Scalar HWDGE queues; abs-max reduces trail each piece.
            # GpSimd's XYZWC reduce is ~1.6x slower than DVE's XY reduce,
            # so GpSimd takes 3 early pieces, DVE the rest (fast tail).
            n_ld = mg_n // 2
            gps_idx = {1, 3, 5}
            n_dve = n_ld - len(gps_idx)
            acc_cols = small.tile([P, n_dve], F32)
            acc_sc = small.tile([P, len(gps_idx)], F32)
            nd = ng = 0
            for g in range(n_ld):
                eng = nc.sync if g % 2 == 0 else nc.scalar
                eng.dma_start(
                    out=xnat[:, 2 * g:2 * g + 2, :],
                    in_=x.ap()[g * 256:(g + 1) * 256, :].rearrange(
                        "(b p) k2 -> p b k2", b=2
                    ),
                )
                if g in gps_idx:
                    nc.gpsimd.tensor_reduce(
                        acc_sc[0:1, ng:ng + 1], xnat[:, 2 * g:2 * g + 2, :],
                        axis=mybir.AxisListType.XYZWC,
                        op=mybir.AluOpType.max,
                        apply_absolute_value=True,
                    )
                    ng += 1
                else:
                    nc.vector.tensor_reduce(
                        acc_cols[:, nd:nd + 1], xnat[:, 2 * g:2 * g + 2, :],
                        axis=mybir.AxisListType.XY,
                        op=mybir.AluOpType.max,
                        apply_absolute_value=True,
                    )
                    nd += 1
            # Combine partial maxima -> one [1,1] scalar on partition 0.
            amax_col = small.tile([P, 1], F32)
            nc.vector.tensor_reduce(
                amax_col, acc_cols[:], axis=mybir.AxisListType.X,
                op=mybir.AluOpType.max,
            )
            amax_d0 = small.tile([P, 1], F32)
            nc.gpsimd.tensor_reduce(
                amax_d0[0:1, :], amax_col, axis=mybir.AxisListType.C,
                op=mybir.AluOpType.max,
            )
            amax_g0 = small.tile([P, 1], F32)
            nc.vector.tensor_reduce(
                amax_g0[0:1, :], acc_sc[0:1, :], axis=mybir.AxisListType.X,
                op=mybir.AluOpType.max,
            )
            amax_all = small.tile([P, 1], F32)
            nc.vector.scalar_tensor_tensor(
                amax_all[0:1, :], amax_d0[0:1, :], 1.0, amax_g0[0:1, :],
                mybir.AluOpType.mult, mybir.AluOpType.max,
            )

            # ---- AllGather amaxes across cores, reduce locally ------------
            cc_in = dram.tile([1], F32)
            cc_out = dram.tile([n_cores], F32, addr_space=cc_addr)
            nc.scalar.dma_start(cc_in[:], amax_all[0:1, 0])
            cci = nc.gpsimd.collective_compute(
                "AllGather",
                mybir.AluOpType.bypass,
                replica_groups=[list(range(n_cores))],
                ins=[cc_in.opt()],
                outs=[cc_out.opt()],
            )

            # ---- Weight + bias loads (Scalar queue).  Explicitly held
            # until the collective trigger so Tile can't hoist them into
            # the x-read window (they'd steal HBM bandwidth); they hide in
            # the CC wait instead.
            for i in range(2):
                n0 = i * (n // 2)
                wi = nc.scalar.dma_start(
                    out=wq[:, :, n0:n0 + n // 2],
                    in_=wq8.ap()[:, n0:n0 + n // 2].rearrange(
                        "(j p) n2 -> p j n2", p=P
                    ),
                )
                tile.add_dep_helper(
                    wi.ins, cci.ins, sync=False,
                    reason="hold weight load out of the x-read window",
                )
            # bias comes host-prepped as [128, 16] ([p, j] = bias[j*128+p]);
            # a flat (j p)->p j load would emit 2048 2-byte descriptors.
            bias16 = small.tile([P, nt_tiles], F16)
            bi = nc.scalar.dma_start(bias16[:], b.ap())
            tile.add_dep_helper(
                bi.ins, cci.ins, sync=False,
                reason="hold bias load out of the x-read window",
            )
            bias32 = small.tile([P, nt_tiles], F32)
            nc.vector.tensor_copy(bias32[:], bias16[:])

            # Readback; inv2s = 224/amax (quant scale), s2 = amax/224
            # (dequant scale) computed on partition 0, then one broadcast
            # of the packed [1,2] pair.
            scal0 = small.tile([P, n_cores], F32)
            nc.scalar.dma_start(scal0[0:1, :], cc_out[:])
            amax1 = small.tile([P, 1], F32)
            nc.vector.tensor_reduce(
                amax1[0:1, :], scal0[0:1, :], axis=mybir.AxisListType.X,
                op=mybir.AluOpType.max,
            )
            sc_pair = small.tile([P, 2], F32)
            inv_amax = small.tile([P, 1], F32)
            nc.vector.reciprocal(inv_amax[0:1, :], amax1[0:1, :])
            nc.vector.tensor_scalar_mul(
                sc_pair[0:1, 0:1], inv_amax[0:1, :], 224.0
            )
            nc.vector.tensor_scalar_mul(
                sc_pair[0:1, 1:2], amax1[0:1, :], 1.0 / 224.0
            )
            sc_bc = small.tile([P, 2], F32)
            nc.gpsimd.partition_broadcast(sc_bc, sc_pair[0:1, :], channels=P)
            inv2s = sc_bc[:, 0:1]
            s2 = sc_bc[:, 1:2]

            # ---- Quantize (natural layout) + packed transposes -----------
            # First 4 blocks split DVE||ScalarE for the fastest GEMM start;
            # later blocks mostly DVE (ScalarE is busy with evictions).
            tr_insts = []

            def emit_block(mg):
                if mg < 4:
                    h = k // 2
                    nc.vector.tensor_scalar(
                        xqn[:, mg, 0:h], xnat[:, mg, 0:h], inv2s, None,
                        mybir.AluOpType.mult,
                    )
                    nc.scalar.activation(
                        xqn[:, mg, h:k], xnat[:, mg, h:k],
                        mybir.ActivationFunctionType.Copy, scale=inv2s,
                    )
                elif mg in (5, 9, 13):
                    nc.scalar.activation(
                        xqn[:, mg, :], xnat[:, mg, :],
                        mybir.ActivationFunctionType.Copy, scale=inv2s,
                    )
                else:
                    nc.vector.tensor_scalar(
                        xqn[:, mg, :], xnat[:, mg, :], inv2s, None,
                        mybir.AluOpType.mult,
                    )
                ti = nc.sync.dma_start(
                    out=xqT[:, :, mg * P:(mg + 1) * P],
                    in_=xqn[:, mg, :].bitcast(F16),
                    transpose=True,
                )
                tr_insts.append(ti)

            # All quant+transpose blocks first in program order so the Sync
            # queue runs loads -> transposes -> evictions (clean transpose
            # burst, no copy interleave); the scheduler pipelines the GEMM
            # in by data deps.
            for mg in range(mg_n):
                emit_block(mg)

            # ---- GEMM (fp8 DoubleRow) + fused eviction -------------------
            out_dmas = []
            for ci, (b0, b1) in enumerate(CHUNK_PLAN):
                m0 = b0 * P
                msz = (b1 - b0) * P
                for nt in range(nt_tiles):
                    ps = psum.tile(
                        [P, msz], F32, tag="ps", bufs=7, name=f"ps_{ci}_{nt}"
                    )
                    for jj in range(kj):
                        rhs = (
                            xqT[:, jj, m0:m0 + msz]
                            .bitcast(FP8)
                            .rearrange("p (m two) -> p two m", two=2)
                        )
                        nc.tensor.matmul(
                            ps[:],
                            lhsT=wq[:, 2 * jj:2 * jj + 2, nt * P:(nt + 1) * P],
                            rhs=rhs,
                            start=(jj == 0),
                            stop=(jj == kj - 1),
                            perf_mode=mybir.MatmulPerfMode.DoubleRow,
                        )
                    ob = ev.tile([P, msz], F16, tag="ob", name=f"ob_{ci}_{nt}")
                    nc.scalar.activation(
                        ob[:], ps[:],
                        mybir.ActivationFunctionType.Identity,
                        bias=bias32[:, nt:nt + 1],
                        scale=s2,
                    )
                    oi = nc.sync.dma_start(
                        out_t.ap()[nt * P:(nt + 1) * P, m0:m0 + msz], ob[:]
                    )
                    out_dmas.append(oi)

            # Order the early output DMAs after the final transpose (xbar
            # transpose || copy hazard); the ob pool absorbs the backlog.
            for oi in out_dmas[:16]:
                tile.add_dep_helper(
                    oi.ins, tr_insts[-1].ins,
                    reason="xbar: evictions after transpose burst",
                )

    nc.compile()
    return nc


_NC_CACHE = {}


def _get_nc():
    if "nc" not in _NC_CACHE:
        _NC_CACHE["nc"] = build_nc()
    return _NC_CACHE["nc"]


def kernel(x, weight, bias):
    x = np.asarray(x, dtype=np.float16).reshape(M, K)
    weight = np.asarray(weight, dtype=np.float16)
    bias = np.asarray(bias, dtype=np.float16)

    nc = _get_nc()
    # Static-weight host prep: quantize (scale 1.0 -> plain RNE cast onto
    # the reference's e4m3fn grid; |w|<240 so bits == TRN fp8e4), transpose
    # to [K, N], and pack rows in k-PAIR order to match the on-chip packed
    # transpose: DRAM row (jj*256 + pr*128 + q) holds k = jj*256 + 2q + pr.
    w8T = weight.astype(np.float32).astype(ml_dtypes.float8_e4m3fn).T
    wq8 = np.ascontiguousarray(
        w8T.reshape(K // 256, 128, 2, N).transpose(0, 2, 1, 3).reshape(K, N)
    )
    bias_pj = np.ascontiguousarray(bias.reshape(N // P, P).T)  # [p, j]
    in_maps = [
        {"x": x[c * MS:(c + 1) * MS], "wq8": wq8, "b": bias_pj}
        for c in range(N_CORES)
    ]
    trace = bool(int(os.environ.get("KERNEL_TRACE", "0")))
    res = run_bass_kernel_spmd(nc, in_maps, list(range(N_CORES)), trace=trace)
    _NC_CACHE["last_result"] = res

    out = np.empty((M, N), dtype=np.float16)
    for c in range(N_CORES):
        out[c * MS:(c + 1) * MS, :] = res.results[c]["out_t"].T
    return out.reshape(B, S, N)
